# revision 23
# baseline (speedup 1.0000x reference)
"""nn_EncoderModel: 2-layer LSTM encoder (B=128, T=512, E=256, H=1024)
on 8 trn2 NeuronCores.

Strategy: hidden-dim model parallelism with a software-pipelined layer-1
(lagged one timestep). Core k owns h-dims [128k, 128(k+1)) of BOTH LSTM
layers for the full batch. Per pipeline tick tau:
  - layer 0 computes step tau:   z0(tau) = onehot @ EWb + h0(tau-1) @ W0h
  - layer 1 computes step tau-1: z1(tau-1) = nh0(tau-1) @ W1a + h1(tau-2) @ W1b
  - ONE AllGather ships {nh0(tau).T, nh1(tau-1).T} = [256, B] bf16 per
    rank (vs 2 fp32 AGs/step in the naive schedule): recv(tau) holds
    h0(tau) in its first half and h1(tau-1) in its second half, exactly
    the operands ticks tau+1 needs.
Weights and the h-transport are bf16 (PE 1 cyc/row, half the AG wire
bytes); gate PSUM accumulation and all cell math stay fp32. Gates are
host-reordered to [f|i|o|j] so each cell needs only 2-3 ACT calls
(sigmoid over a contiguous [B,384]/[B,256] span + tanh). The embedding
lookup is folded into layer 0 via onehot(tok) @ (emb @ W0x + b0), with
layer-0's forget bias folded in too. cell0 skips the length-mask blend
entirely (cell1's frozen h1 makes post-length h0 irrelevant); cell1
keeps it. Output = final h1 state = output at the last valid step.
"""

from contextlib import ExitStack

import ml_dtypes
import numpy as np

import concourse.bass as bass
import concourse.mybir as mybir
import concourse.tile as tile
from concourse import bacc
from concourse.bass_utils import run_bass_kernel_spmd

F32 = mybir.dt.float32
BF16 = mybir.dt.bfloat16
AF = mybir.ActivationFunctionType
ALU = mybir.AluOpType

B = 128      # batch (full, on every core)
E = 256      # embedding dim
H = 1024     # hidden
V = 128      # vocab
T = 512      # timesteps
HSL = 128    # hidden slice per core
G = 4 * HSL  # gate cols per core = 512
NCORES = 8


def _host_prep(inputs):
    """Slice/transform full inputs into 8 per-core input maps."""
    ib = np.asarray(inputs["input_batch"])            # [B, T] int32
    lens = np.asarray(inputs["input_lengths"])        # [B]
    emb = np.asarray(inputs["char_embeddings"], dtype=np.float32)  # [V, E]
    W0 = np.asarray(inputs["W0"], dtype=np.float32)   # [E+H, 4H]
    b0 = np.asarray(inputs["b0"], dtype=np.float32)
    W1 = np.asarray(inputs["W1"], dtype=np.float32)   # [2H, 4H]
    b1 = np.asarray(inputs["b1"], dtype=np.float32)

    GO = (2, 0, 3, 1)  # gate order: f, i, o, j (i,j,f,o in W layout)

    def gate_cols(W, k):
        return np.concatenate(
            [W[:, g * H + k * HSL: g * H + (k + 1) * HSL] for g in GO],
            axis=1,
        )

    def gate_cols_b(b, k):
        return np.concatenate(
            [b[g * H + k * HSL: g * H + (k + 1) * HSL] for g in GO]
        )

    tok_f32 = ib[:, :T].astype(np.float32)            # [B, T]
    mask = (np.arange(T)[None, :] < lens[:, None]).astype(np.float32)
    iota_free = np.tile(np.arange(V, dtype=np.float32)[None, :], (B, 1))
    ident = np.eye(128, dtype=np.float32)

    in_maps = []
    for k in range(NCORES):
        W0c = gate_cols(W0, k)                        # [E+H, 512]
        b0c = gate_cols_b(b0, k).copy()               # [512]
        b0c[0:HSL] += 1.0                  # forget bias layer 0
        ewb = emb @ W0c[:E] + b0c[None, :]            # [V, 512]
        w0h = W0c[E:]                                 # [1024, 512]
        w0h_t = np.concatenate(
            [w0h[j * 128: (j + 1) * 128] for j in range(8)], axis=1)
        W1c = gate_cols(W1, k)                        # [2048, 512]
        b1c = gate_cols_b(b1, k)
        # k-tile blocks: 0..7 = nh0 rows, 8..15 = h1 rows
        w1_t = np.concatenate(
            [W1c[j * 128: (j + 1) * 128] for j in range(16)], axis=1)
        b1_full = np.tile(b1c[None, :], (128, 1)).astype(np.float32)
        in_maps.append({
            "ewb": ewb.astype(ml_dtypes.bfloat16),
            "w0h": w0h_t.astype(ml_dtypes.bfloat16),
            "w1": w1_t.astype(ml_dtypes.bfloat16),
            "b1full": b1_full,
            "tok": tok_f32,
            "mask": mask,
            "iotaf": iota_free,
            "ident": ident,
        })
    has_b1 = bool(np.any(b1 != 0.0))
    return in_maps, has_b1


def build_kernel(has_b1=False, n_steps=T):
    """Build + compile the SPMD Bass kernel for all 8 cores."""
    nc = bacc.Bacc("TRN2", target_bir_lowering=False, debug=False,
                   num_devices=NCORES)

    # ---- I/O (weights + h-transport in bf16) ----
    d_ewb = nc.dram_tensor("ewb", [V, G], BF16, kind="ExternalInput")
    d_w0h = nc.dram_tensor("w0h", [128, 8 * G], BF16, kind="ExternalInput")
    d_w1 = nc.dram_tensor("w1", [128, 16 * G], BF16, kind="ExternalInput")
    d_b1 = nc.dram_tensor("b1full", [128, G], F32, kind="ExternalInput")
    d_tok = nc.dram_tensor("tok", [B, T], F32, kind="ExternalInput")
    d_mask = nc.dram_tensor("mask", [B, T], F32, kind="ExternalInput")
    d_iota = nc.dram_tensor("iotaf", [B, V], F32, kind="ExternalInput")
    d_ident = nc.dram_tensor("ident", [128, 128], F32, kind="ExternalInput")
    d_out = nc.dram_tensor("out", [B, HSL], F32, kind="ExternalOutput")

    # ---- persistent SBUF ----
    sb_ewb = nc.alloc_sbuf_tensor("sb_ewb", [V, G], BF16)
    sb_w0h = nc.alloc_sbuf_tensor("sb_w0h", [128, 8 * G], BF16)
    sb_w1 = nc.alloc_sbuf_tensor("sb_w1", [128, 16 * G], BF16)
    sb_b1 = nc.alloc_sbuf_tensor("sb_b1", [128, G], F32)
    sb_tok = nc.alloc_sbuf_tensor("sb_tok", [B, T], F32)
    sb_mask = nc.alloc_sbuf_tensor("sb_mask", [B, T], F32)
    sb_iota = nc.alloc_sbuf_tensor("sb_iota", [B, V], F32)
    sb_ident = nc.alloc_sbuf_tensor("sb_ident", [128, 128], F32)
    c0 = nc.alloc_sbuf_tensor("c0", [B, HSL], F32)
    c1 = nc.alloc_sbuf_tensor("c1", [B, HSL], F32)
    h0bt = nc.alloc_sbuf_tensor("h0bt", [B, HSL], F32)
    h1bt = nc.alloc_sbuf_tensor("h1bt", [B, HSL], F32)

    with tile.TileContext(nc) as tc, ExitStack() as ctx:
        # ---- load weights/constants ----
        for sb, d in [(sb_ewb, d_ewb), (sb_w0h, d_w0h), (sb_w1, d_w1),
                      (sb_b1, d_b1), (sb_tok, d_tok), (sb_mask, d_mask),
                      (sb_iota, d_iota), (sb_ident, d_ident)]:
            nc.sync.dma_start(sb[:], d[:])
        for st in (c0, c1, h0bt, h1bt):
            nc.vector.memset(st[:], 0.0)

        # ---- pools ----
        ps_z0 = ctx.enter_context(tc.tile_pool(name="psz0", bufs=2, space="PSUM"))
        ps_z1 = ctx.enter_context(tc.tile_pool(name="psz1", bufs=2, space="PSUM"))
        ps_oh = ctx.enter_context(tc.tile_pool(name="psoh", bufs=2, space="PSUM"))
        ps_tp = ctx.enter_context(tc.tile_pool(name="pstp", bufs=2, space="PSUM"))
        pool = ctx.enter_context(tc.tile_pool(name="work", bufs=3))
        rpool = ctx.enter_context(tc.tile_pool(name="recv", bufs=3))
        dram = ctx.enter_context(tc.tile_pool(name="dram", bufs=2, space="DRAM"))

        def cell0(zap):
            """Unmasked LSTM cell on [B, G] gates; updates c0/h0bt in place."""
            fio = pool.tile([B, 3 * HSL], F32, tag="fio0")
            tanj = pool.tile([B, HSL], F32, tag="tanj0")
            nc.scalar.activation(fio[:], zap[:, 0:384], AF.Sigmoid)
            nc.scalar.activation(tanj[:], zap[:, 384:512], AF.Tanh)
            ij = pool.tile([B, HSL], F32, tag="ij0")
            nc.vector.tensor_mul(c0[:], c0[:], fio[:, 0:128])
            nc.vector.tensor_mul(ij[:], fio[:, 128:256], tanj[:])
            nc.vector.tensor_add(c0[:], c0[:], ij[:])
            tanc = pool.tile([B, HSL], F32, tag="tanc0")
            nc.scalar.activation(tanc[:], c0[:], AF.Tanh)
            nc.vector.tensor_mul(h0bt[:], tanc[:], fio[:, 256:384])

        def cell1(zap, t):
            """Masked LSTM cell on [B, G] gates; updates c1/h1bt in place."""
            m = sb_mask[:, t:t + 1]
            sigf = pool.tile([B, HSL], F32, tag="sigf1")
            sio = pool.tile([B, 2 * HSL], F32, tag="sio1")
            tanj = pool.tile([B, HSL], F32, tag="tanj1")
            nc.scalar.activation(sigf[:], zap[:, 0:128], AF.Sigmoid,
                                 bias=1.0)
            nc.scalar.activation(sio[:], zap[:, 128:384], AF.Sigmoid)
            nc.scalar.activation(tanj[:], zap[:, 384:512], AF.Tanh)
            cf = pool.tile([B, HSL], F32, tag="cf1")
            ij = pool.tile([B, HSL], F32, tag="ij1")
            craw = pool.tile([B, HSL], F32, tag="craw1")
            nc.vector.tensor_mul(cf[:], c1[:], sigf[:])
            nc.vector.tensor_mul(ij[:], sio[:, 0:128], tanj[:])
            nc.vector.tensor_add(craw[:], cf[:], ij[:])
            # masked blend: c1 += m * (craw - c1)
            cd = pool.tile([B, HSL], F32, tag="cd1")
            nc.vector.tensor_sub(cd[:], craw[:], c1[:])
            nc.vector.tensor_scalar(cd[:], cd[:], m, None, ALU.mult)
            nc.vector.tensor_add(c1[:], c1[:], cd[:])
            tanc = pool.tile([B, HSL], F32, tag="tanc1")
            nc.scalar.activation(tanc[:], c1[:], AF.Tanh)
            nhr = pool.tile([B, HSL], F32, tag="nhr1")
            nc.vector.tensor_mul(nhr[:], tanc[:], sio[:, 128:256])
            hd = pool.tile([B, HSL], F32, tag="hd1")
            nc.vector.tensor_sub(hd[:], nhr[:], h1bt[:])
            nc.vector.tensor_scalar(hd[:], hd[:], m, None, ALU.mult)
            nc.vector.tensor_add(h1bt[:], h1bt[:], hd[:])

        recv_hist = [None, None]  # [recv(tau-2), recv(tau-1)]

        for tau in range(n_steps + 1):
            do_l0 = tau < n_steps
            do_l1 = tau >= 1
            # recv(tau-1): h0(tau-1) blocks in first half [0:1024],
            # h1(tau-2) blocks in second half [1024:2048]
            r_prev = recv_hist[1]

            # ---- layer 1 part A: h1(tau-2) contribution ----
            if do_l1:
                z1 = ps_z1.tile([B, G], F32, tag="z1", name="z1")
            else:
                z1 = None
            # ---- layer 1 part A: h1(tau-2) contribution ----
            if tau >= 2:
                for j in range(8):
                    nc.tensor.matmul(
                        z1[:], r_prev[:, NCORES * HSL + j * 128:NCORES * HSL + (j + 1) * 128],
                        sb_w1[:, (8 + j) * G:(9 + j) * G],
                        start=(j == 0), stop=False, skip_group_check=True)

            if do_l0:
                # ---- one-hot for x_tau -> lhsT [V, B] ----
                ohbt = pool.tile([B, V], F32, tag="ohbt")
                nc.vector.tensor_scalar(
                    ohbt[:], sb_iota[:], sb_tok[:, tau:tau + 1], None,
                    ALU.is_equal)
                poh = ps_oh.tile([V, B], F32, tag="poh")
                nc.tensor.transpose(poh[:], ohbt[:], sb_ident[:])
                ohT = pool.tile([V, B], BF16, tag="ohT")
                nc.vector.tensor_copy(ohT[:], poh[:])

                # ---- layer 0: z0 = onehot @ EWb (+ h0(tau-1) @ W0h) ----
                z0 = ps_z0.tile([B, G], F32, tag="z0")
                nc.tensor.matmul(z0[:], ohT[:], sb_ewb[:],
                                 start=True, stop=(tau == 0),
                                 skip_group_check=True)
                if tau > 0:
                    for j in range(8):
                        nc.tensor.matmul(
                            z0[:], r_prev[:, j * 128:(j + 1) * 128],
                            sb_w0h[:, j * G:(j + 1) * G],
                            start=False, stop=(j == 7), skip_group_check=True)
                cell0(z0)

            # ---- stage nh0(tau).T into cin rows 0:128 ----
            if do_l0:
                cin = dram.tile([2 * HSL, B], BF16, tag="cin")
                tp0 = ps_tp.tile([HSL, B], F32, tag="tp")
                nc.tensor.transpose(tp0[:], h0bt[:], sb_ident[:])
                stg0 = pool.tile([HSL, B], BF16, tag="stg0")
                nc.vector.tensor_copy(stg0[:], tp0[:])
                nc.sync.dma_start(cin[0:HSL, :], stg0[:])

            # ---- layer 1 part B: nh0(tau-1) contribution ----
            if do_l1:
                for j in range(8):
                    nc.tensor.matmul(
                        z1[:], r_prev[:, j * 128:(j + 1) * 128],
                        sb_w1[:, j * G:(j + 1) * G],
                        start=(tau < 2 and j == 0), stop=(j == 7),
                        skip_group_check=True)
                if has_b1:
                    zb = pool.tile([B, G], F32, tag="zb")
                    nc.vector.tensor_add(zb[:], z1[:], sb_b1[:])
                    z1ap = zb
                else:
                    z1ap = z1
                cell1(z1ap, tau - 1)

            # ---- stage nh1(tau-1).T into cin rows 128:256, then AG ----
            if do_l0:
                tp1 = ps_tp.tile([HSL, B], F32, tag="tp")
                nc.tensor.transpose(tp1[:], h1bt[:], sb_ident[:])
                stg1 = pool.tile([HSL, B], BF16, tag="stg1")
                nc.vector.tensor_copy(stg1[:], tp1[:])
                nc.sync.dma_start(cin[HSL:2 * HSL, :], stg1[:])

                cout = dram.tile([NCORES * 2 * HSL, B], BF16, tag="cout",
                                 addr_space="Shared")
                nc.gpsimd.collective_compute(
                    "AllGather", ALU.bypass,
                    replica_groups=[list(range(NCORES))],
                    ins=[cin.opt()], outs=[cout.opt()],
                )
                recv = rpool.tile([128, NCORES * 2 * HSL], BF16, tag="recv")
                csrc = cout[:].rearrange("(j q p) c -> q p j c", q=2, p=128)
                nc.sync.dma_start(
                    recv[:, 0:NCORES * HSL].rearrange("p (j c) -> p j c", c=B),
                    csrc[0])
                nc.scalar.dma_start(
                    recv[:, NCORES * HSL:].rearrange("p (j c) -> p j c", c=B),
                    csrc[1])
                recv_hist = [recv_hist[1], recv]

        # ---- output ----
        nc.sync.dma_start(d_out[:], h1bt[:])

    nc.compile()
    return nc


_CACHE = {}


def kernel(**inputs) -> np.ndarray:
    """Full-input entry point: returns [B, H] fp32 encoder output."""
    in_maps, has_b1 = _host_prep(inputs)
    key = ("nc", has_b1)
    if key not in _CACHE:
        _CACHE[key] = build_kernel(has_b1=has_b1)
    nc = _CACHE[key]
    res = run_bass_kernel_spmd(nc, in_maps, core_ids=list(range(NCORES)))
    out = np.concatenate(
        [res.results[k]["out"] for k in range(NCORES)], axis=1)
    return out.astype(np.float32)
